# revision 25
# baseline (speedup 1.0000x reference)
"""LocallyConnected2d (B=8, C_in=32, 48x48, C_out=32, 3x3, pad 1) on 8 trn2 cores.

Strategy: shard the spatial-location axis L = H*W across cores (6 image rows
each). Per location l the op is an (8x288)@(288x32) GEMM with location-unique
weights; weight streaming (42.5 MB fp16 total) dominates -> memory-bound.

v6 design (vs v1 baseline at 63 us):
  - Bias is folded into the matmul as a 97th contraction row: x gets a
    constant-1.0 partition row 96 (device memset); weight row 96 holds
    bias[l, o] in each kh=0 block and zeros in kh=1,2 blocks, so EVERY
    matmul uses K=97 (uniform LDWEIGHTS shape keeps the LDW pipelined
    behind the matmul stream; mixed 96/97 serializes it, +8 cyc/pair).
    This deletes all 72 bias matmuls of the baseline.
  - All weights live in ONE SBUF tile [97, 55296B]: 9 x [96, 6144B-row]
    DMAs (sub-range slices) stream back-to-back on the sync HWDGE queue
    (hardware completion; the gpsimd SWDGE queue serializes gen->drain at
    ~3.7us/tile). The bias row is ONE [1, 55296B] DMA issued first.
    Descriptor counts stay multiples of 16 -- odd-count DMAs collapse onto
    a single DMA engine. x/stat is split in two DMAs on the scalar queue
    (concurrent with the weight stream) so the PE can start on tile 0 as
    early as possible; output stores go to gpsimd.
  - Per weight tile (32 locations = 2 groups) matmuls accumulate in a
    2-bank PSUM tile [128, 1024]; 4 copies (one per PE column group j) move
    ps[32j:32j+8] for both groups to SBUF fp16, split across the vector and
    scalar engines. Output is [128, 1536B-row] fp16 chunks, unscrambled and
    upcast on the host.
  - Contraction (d=288) is split into 3 kh-rounds of K=96=(3 kw x 32 c),
    PSUM-accumulated; x halo slice lives in SBUF replicated 3x with kw
    column shifts, so any patch is a plain strided AP slice.
"""

import numpy as np

import concourse.bacc as bacc
import concourse.tile as tile
from concourse import mybir
from concourse.bass_utils import run_bass_kernel_spmd

B, C_IN, H, W = 8, 32, 48, 48
C_OUT = 32
N_CORES = 8
RP = H // N_CORES  # rows per core (6)
LP = RP * W  # locations per core (288)
NGRP = LP // 16  # 16-loc output groups per core (18)

DT16 = True  # fp16 operand path (halves weight traffic)
DT = mybir.dt.float16 if DT16 else mybir.dt.float32
NPDT = np.float16 if DT16 else np.float32
F32 = mybir.dt.float32

TL = 16  # locations per weight tile -> [96, 3072B] DMAs
NT = LP // TL  # 18 weight tiles
WTF = 3 * TL * C_OUT  # weight tile free elems (3072)
XF = (RP + 2) * W * B  # x3 free size (3072)

_nc = None


def _build():
    nc = bacc.Bacc(
        "TRN2", target_bir_lowering=False, debug=False, num_devices=N_CORES
    )
    stat = nc.dram_tensor("stat", [96, XF], DT, kind="ExternalInput")
    wds = [
        nc.dram_tensor(f"w{t}", [96, WTF], DT, kind="ExternalInput")
        for t in range(NT)
    ]
    biasd = nc.dram_tensor("biasd", [1, NT * WTF], DT, kind="ExternalInput")
    out = nc.dram_tensor("out", [128, NGRP * 32], DT, kind="ExternalOutput")

    with tile.TileContext(nc) as tc:
        with (
            tc.tile_pool(name="xpool", bufs=1) as xpool,
            tc.tile_pool(name="wpool", bufs=1) as wpool,
            tc.tile_pool(name="opool", bufs=1) as opool,
            tc.tile_pool(name="pspool", bufs=8, space="PSUM") as pspool,
        ):
            stat_sb = xpool.tile([97, XF], DT, tag="stat")
            wt_all = wpool.tile([97, NT * WTF], DT, tag="wt")

            # x (2 chunks) + bias row on scalar, concurrent with the weight
            # stream; sync carries ONLY the 9 weight tiles back-to-back
            nc.gpsimd.memset(stat_sb[96:97, :], 1.0)
            nc.scalar.dma_start(stat_sb[0:96, 0 : 3 * W * B], stat[:, 0 : 3 * W * B])
            nc.scalar.dma_start(wt_all[96:97, :], biasd[:, :])
            nc.scalar.dma_start(stat_sb[0:96, 3 * W * B : XF], stat[:, 3 * W * B : XF])
            # (order: statA -> bias -> statB; statB is only needed from gi=3)
            for t in range(NT):
                nc.sync.dma_start(
                    wt_all[0:96, t * WTF : (t + 1) * WTF], wds[t][:, :]
                )

            out_sb = opool.tile([128, NGRP * 32], DT)

            # Stationary = W [97, 32] (LDW streams 32 cols, double-buffered
            # across the 4 PE column groups), moving = x [97, 8] (8-row
            # matmul). Out = [32 o-partitions at 32j, 8 b-free]: psum per
            # 16-loc group is only [128, 32] ((j,o) x (m,b)).
            for t in range(NT):
                ps = pspool.tile([128, 512], F32)
                rl, qg = divmod(t, 3)
                for m in range(4):
                    for kh in range(3):
                        for j in range(4):
                            q = qg * 16 + m * 4 + j
                            ll = m * 4 + j
                            off = ((rl + kh) * W + q) * B
                            wo = t * WTF + (kh * TL + ll) * 32
                            nc.tensor.matmul(
                                ps[32 * j : 32 * j + 32, m * 8 : m * 8 + B],
                                wt_all[0:97, wo : wo + 32],
                                stat_sb[0:97, off : off + B],
                                start=(kh == 0),
                                stop=(kh == 2),
                                skip_group_check=True,
                                tile_position=(0, 32 * j),
                            )
                # one copy per tile moves the group's psum to SBUF as fp16
                nc.vector.tensor_copy(
                    out_sb[:, 32 * t : 32 * (t + 1)], ps[0:128, 0:32]
                )
                # two stores only (keeps the DMA/semaphore count low)
                if t == 8:
                    nc.gpsimd.dma_start(out[:, 0:288], out_sb[:, 0:288])
                elif t == 17:
                    nc.gpsimd.dma_start(out[:, 288:576], out_sb[:, 288:576])
    nc.compile()
    return nc


def _shard(inputs):
    x = np.asarray(inputs["x"], np.float32)
    weight = np.asarray(inputs["weight"], np.float32)[0]  # (288, L, 32)
    bias = np.asarray(inputs["bias"], np.float32)[0]  # (32, 48, 48)
    xp = np.pad(x, ((0, 0), (0, 0), (1, 1), (1, 1)))  # (b, c, 50, 50)
    bias_t = bias.reshape(C_OUT, H * W).T  # (L, C_OUT)
    wflat = weight.reshape(C_IN, 3, 3, H * W, C_OUT)  # (c, kh, kw, l, o)

    in_maps = []
    for k in range(N_CORES):
        r0 = RP * k
        l0 = LP * k

        x3h = np.empty((3, C_IN, RP + 2, W, B), np.float32)
        for kw in range(3):
            x3h[kw] = xp[:, :, r0 : r0 + RP + 2, kw : kw + W].transpose(1, 2, 3, 0)
        stat = x3h.reshape(96, XF).astype(NPDT)

        # W tiles: [(kw c), (kh, lg, o)]; bias row separately, kh=0 block only
        wk = wflat[:, :, :, l0 : l0 + LP, :]  # (c, kh, kw, LP, o)
        wall = wk.transpose(2, 0, 1, 3, 4).reshape(96, 3, LP, C_OUT)
        m = {"stat": stat}
        for t in range(NT):
            m[f"w{t}"] = np.ascontiguousarray(
                wall[:, :, TL * t : TL * (t + 1), :].reshape(96, WTF)
            ).astype(NPDT)
        brow = np.zeros((NT, 3, TL, C_OUT), NPDT)
        brow[:, 0] = bias_t[l0 : l0 + LP, :].reshape(NT, TL, C_OUT).astype(NPDT)
        m["biasd"] = brow.reshape(1, NT * WTF)
        in_maps.append(m)
    return in_maps


def _get_nc():
    global _nc
    if _nc is None:
        _nc = _build()
    return _nc


def _gather(results):
    # out row 32j+o holds y[b, o, r, q] at col gi*32 + m*8 + b,
    # with r = gi//3, q = (gi%3)*16 + m*4 + j
    y = np.empty((B, C_OUT, H, W), np.float32)
    for k in range(N_CORES):
        arr = results[k]["out"].astype(np.float32)
        arr = arr.reshape(4, C_OUT, NGRP, 4, B)  # (j, o, gi, m, b)
        arr = arr.transpose(4, 1, 2, 3, 0)  # (b, o, gi, m, j)
        arr = arr.reshape(B, C_OUT, RP, 3, 4, 4)  # (b, o, r, qg, m, j)
        y[:, :, RP * k : RP * (k + 1), :] = arr.reshape(B, C_OUT, RP, W)
    return y


def kernel(**inputs):
    nc = _get_nc()
    res = run_bass_kernel_spmd(nc, _shard(inputs), list(range(N_CORES)))
    return _gather(res.results)


# revision 28
# speedup vs baseline: 1.1931x; 1.1931x over previous
"""LocallyConnected2d (B=8, C_in=32, 48x48, C_out=32, 3x3, pad 1) on 8 trn2 cores.

Strategy: shard the spatial-location axis L = H*W across cores (6 image rows
each). Per location l the op is an (8x288)@(288x32) GEMM with location-unique
weights; weight streaming (42.5 MB fp16 total) dominates -> memory-bound.

v6 design (vs v1 baseline at 63 us):
  - Bias is folded into the matmul as a 97th contraction row: x gets a
    constant-1.0 partition row 96 (device memset); weight row 96 holds
    bias[l, o] in each kh=0 block and zeros in kh=1,2 blocks, so EVERY
    matmul uses K=97 (uniform LDWEIGHTS shape keeps the LDW pipelined
    behind the matmul stream; mixed 96/97 serializes it, +8 cyc/pair).
    This deletes all 72 bias matmuls of the baseline.
  - All weights live in ONE SBUF tile [97, 55296B]: 9 x [96, 6144B-row]
    DMAs (sub-range slices) stream back-to-back on the sync HWDGE queue
    (hardware completion; the gpsimd SWDGE queue serializes gen->drain at
    ~3.7us/tile). The bias row is ONE [1, 55296B] DMA issued first.
    Descriptor counts stay multiples of 16 -- odd-count DMAs collapse onto
    a single DMA engine. x/stat is split in two DMAs on the scalar queue
    (concurrent with the weight stream) so the PE can start on tile 0 as
    early as possible; output stores go to gpsimd.
  - Per weight tile (32 locations = 2 groups) matmuls accumulate in a
    2-bank PSUM tile [128, 1024]; 4 copies (one per PE column group j) move
    ps[32j:32j+8] for both groups to SBUF fp16, split across the vector and
    scalar engines. Output is [128, 1536B-row] fp16 chunks, unscrambled and
    upcast on the host.
  - Contraction (d=288) is split into 3 kh-rounds of K=96=(3 kw x 32 c),
    PSUM-accumulated; x halo slice lives in SBUF replicated 3x with kw
    column shifts, so any patch is a plain strided AP slice.
"""

import numpy as np

import concourse.bacc as bacc
import concourse.tile as tile
from concourse import mybir
from concourse.bass_utils import run_bass_kernel_spmd

B, C_IN, H, W = 8, 32, 48, 48
C_OUT = 32
N_CORES = 8
RP = H // N_CORES  # rows per core (6)
LP = RP * W  # locations per core (288)
NGRP = LP // 16  # 16-loc output groups per core (18)

DT16 = True  # fp16 operand path (halves weight traffic)
DT = mybir.dt.float16 if DT16 else mybir.dt.float32
NPDT = np.float16 if DT16 else np.float32
F32 = mybir.dt.float32

TL = 32  # locations per weight tile -> [96, 6144B] DMAs
NT = LP // TL  # 9 weight tiles
WTF = 3 * TL * C_OUT  # weight tile free elems (3072)
XF = (RP + 2) * W * B  # x3 free size (3072)

_nc = None


def _build():
    nc = bacc.Bacc(
        "TRN2", target_bir_lowering=False, debug=False, num_devices=N_CORES
    )
    stat = nc.dram_tensor("stat", [96, XF], DT, kind="ExternalInput")
    wds = [
        nc.dram_tensor(f"w{t}", [96, WTF], DT, kind="ExternalInput")
        for t in range(NT)
    ]
    biasd = nc.dram_tensor("biasd", [1, NT * WTF], DT, kind="ExternalInput")
    out = nc.dram_tensor("out", [128, NGRP * 128], DT, kind="ExternalOutput")

    with tile.TileContext(nc) as tc:
        with (
            tc.tile_pool(name="xpool", bufs=1) as xpool,
            tc.tile_pool(name="wpool", bufs=1) as wpool,
            tc.tile_pool(name="opool", bufs=1) as opool,
            tc.tile_pool(name="pspool", bufs=4, space="PSUM") as pspool,
        ):
            stat_sb = xpool.tile([97, XF], DT, tag="stat")
            wt_all = wpool.tile([97, NT * WTF], DT, tag="wt")

            # x (2 chunks) on scalar queue, concurrent with the weight stream
            nc.scalar.dma_start(stat_sb[0:96, 0 : 3 * W * B], stat[:, 0 : 3 * W * B])
            nc.scalar.dma_start(stat_sb[0:96, 3 * W * B : XF], stat[:, 3 * W * B : XF])
            nc.gpsimd.memset(stat_sb[96:97, :], 1.0)

            # bias row first, then the 9 weight tiles back-to-back on sync
            nc.sync.dma_start(wt_all[96:97, :], biasd[:, :])
            for t in range(NT):
                nc.sync.dma_start(
                    wt_all[0:96, t * WTF : (t + 1) * WTF], wds[t][:, :]
                )

            out_sb = opool.tile([128, NGRP * 128], DT)

            for t in range(NT):
                ps = pspool.tile([128, 1024], F32)
                for gl in range(2):
                    gi = 2 * t + gl
                    rl, qg = divmod(gi, 3)
                    for m in range(4):
                        for kh in range(3):
                            for j in range(4):
                                q = qg * 16 + m * 4 + j
                                ll = gl * 16 + m * 4 + j
                                off = ((rl + kh) * W + q) * B
                                wo = t * WTF + (kh * TL + ll) * 32
                                nc.tensor.matmul(
                                    ps[
                                        32 * j : 32 * j + B,
                                        gl * 512 + m * 32 : gl * 512 + (m + 1) * 32,
                                    ],
                                    stat_sb[0:97, off : off + B],
                                    wt_all[0:97, wo : wo + 32],
                                    start=(kh == 0),
                                    stop=(kh == 2),
                                    skip_group_check=True,
                                    tile_position=(0, 32 * j),
                                )
                # move both groups' psum to SBUF; split j over vector/scalar
                for j in range(4):
                    src = ps[32 * j : 32 * j + 32, 0:1024].rearrange(
                        "p (g f) -> p g f", g=2
                    )[:, :, 0:128]
                    dst = out_sb[
                        32 * j : 32 * j + 32, 256 * t : 256 * t + 256
                    ].rearrange("p (g f) -> p g f", g=2)
                    if j < 2:
                        nc.vector.tensor_copy(dst, src)
                    else:
                        nc.scalar.copy(dst, src)
                if t % 3 == 2:
                    s = t // 3
                    nc.gpsimd.dma_start(
                        out[:, 768 * s : 768 * (s + 1)],
                        out_sb[:, 768 * s : 768 * (s + 1)],
                    )
    nc.compile()
    return nc


def _shard(inputs):
    x = np.asarray(inputs["x"], np.float32)
    weight = np.asarray(inputs["weight"], np.float32)[0]  # (288, L, 32)
    bias = np.asarray(inputs["bias"], np.float32)[0]  # (32, 48, 48)
    xp = np.pad(x, ((0, 0), (0, 0), (1, 1), (1, 1)))  # (b, c, 50, 50)
    bias_t = bias.reshape(C_OUT, H * W).T  # (L, C_OUT)
    wflat = weight.reshape(C_IN, 3, 3, H * W, C_OUT)  # (c, kh, kw, l, o)

    in_maps = []
    for k in range(N_CORES):
        r0 = RP * k
        l0 = LP * k

        x3h = np.empty((3, C_IN, RP + 2, W, B), np.float32)
        for kw in range(3):
            x3h[kw] = xp[:, :, r0 : r0 + RP + 2, kw : kw + W].transpose(1, 2, 3, 0)
        stat = x3h.reshape(96, XF).astype(NPDT)

        # W tiles: [(kw c), (kh, lg, o)]; bias row separately, kh=0 block only
        wk = wflat[:, :, :, l0 : l0 + LP, :]  # (c, kh, kw, LP, o)
        wall = wk.transpose(2, 0, 1, 3, 4).reshape(96, 3, LP, C_OUT)
        m = {"stat": stat}
        for t in range(NT):
            m[f"w{t}"] = np.ascontiguousarray(
                wall[:, :, TL * t : TL * (t + 1), :].reshape(96, WTF)
            ).astype(NPDT)
        brow = np.zeros((NT, 3, TL, C_OUT), NPDT)
        brow[:, 0] = bias_t[l0 : l0 + LP, :].reshape(NT, TL, C_OUT).astype(NPDT)
        m["biasd"] = brow.reshape(1, NT * WTF)
        in_maps.append(m)
    return in_maps


def _get_nc():
    global _nc
    if _nc is None:
        _nc = _build()
    return _nc


def _gather(results):
    # out row 32j+b (b<8) holds y[b, o, r, q] at col gi*128 + m*32 + o,
    # with r = gi//3, q = (gi%3)*16 + m*4 + j
    y = np.empty((B, C_OUT, H, W), np.float32)
    for k in range(N_CORES):
        arr = results[k]["out"].astype(np.float32)
        arr = arr.reshape(4, 32, NGRP, 4, C_OUT)[:, 0:B]  # (j, b, gi, m, o)
        arr = arr.transpose(1, 4, 2, 3, 0)  # (b, o, gi, m, j)
        arr = arr.reshape(B, C_OUT, RP, 3, 4, 4)  # (b, o, r, qg, m, j)
        y[:, :, RP * k : RP * (k + 1), :] = arr.reshape(B, C_OUT, RP, W)
    return y


def kernel(**inputs):
    nc = _get_nc()
    res = run_bass_kernel_spmd(nc, _shard(inputs), list(range(N_CORES)))
    return _gather(res.results)


# revision 29
# speedup vs baseline: 1.1933x; 1.0002x over previous
"""LocallyConnected2d (B=8, C_in=32, 48x48, C_out=32, 3x3, pad 1) on 8 trn2 cores.

Strategy: shard the spatial-location axis L = H*W across cores (6 image rows
each). Per location l the op is an (8x288)@(288x32) GEMM with location-unique
weights; weight streaming (42.5 MB fp16 total) dominates -> memory-bound.

v6 design (vs v1 baseline at 63 us):
  - Bias is folded into the matmul as a 97th contraction row: x gets a
    constant-1.0 partition row 96 (device memset); weight row 96 holds
    bias[l, o] in each kh=0 block and zeros in kh=1,2 blocks, so EVERY
    matmul uses K=97 (uniform LDWEIGHTS shape keeps the LDW pipelined
    behind the matmul stream; mixed 96/97 serializes it, +8 cyc/pair).
    This deletes all 72 bias matmuls of the baseline.
  - All weights live in ONE SBUF tile [97, 55296B]: 9 x [96, 6144B-row]
    DMAs (sub-range slices) stream back-to-back on the sync HWDGE queue
    (hardware completion; the gpsimd SWDGE queue serializes gen->drain at
    ~3.7us/tile). The bias row is ONE [1, 55296B] DMA issued first.
    Descriptor counts stay multiples of 16 -- odd-count DMAs collapse onto
    a single DMA engine. x/stat is split in two DMAs on the scalar queue
    (concurrent with the weight stream) so the PE can start on tile 0 as
    early as possible; output stores go to gpsimd.
  - Per weight tile (32 locations = 2 groups) matmuls accumulate in a
    2-bank PSUM tile [128, 1024]; 4 copies (one per PE column group j) move
    ps[32j:32j+8] for both groups to SBUF fp16, split across the vector and
    scalar engines. Output is [128, 1536B-row] fp16 chunks, unscrambled and
    upcast on the host.
  - Contraction (d=288) is split into 3 kh-rounds of K=96=(3 kw x 32 c),
    PSUM-accumulated; x halo slice lives in SBUF replicated 3x with kw
    column shifts, so any patch is a plain strided AP slice.
"""

import numpy as np

import concourse.bacc as bacc
import concourse.tile as tile
from concourse import mybir
from concourse.bass_utils import run_bass_kernel_spmd

B, C_IN, H, W = 8, 32, 48, 48
C_OUT = 32
N_CORES = 8
RP = H // N_CORES  # rows per core (6)
LP = RP * W  # locations per core (288)
NGRP = LP // 16  # 16-loc output groups per core (18)

DT16 = True  # fp16 operand path (halves weight traffic)
DT = mybir.dt.float16 if DT16 else mybir.dt.float32
NPDT = np.float16 if DT16 else np.float32
F32 = mybir.dt.float32

TL = 32  # locations per weight tile -> [96, 6144B] DMAs
NT = LP // TL  # 9 weight tiles
WTF = 3 * TL * C_OUT  # weight tile free elems (3072)
XF = (RP + 2) * W * B  # x3 free size (3072)

_nc = None


def _build():
    nc = bacc.Bacc(
        "TRN2", target_bir_lowering=False, debug=False, num_devices=N_CORES
    )
    stat = nc.dram_tensor("stat", [96, XF], DT, kind="ExternalInput")
    wds = [
        nc.dram_tensor(f"w{t}", [96, WTF], DT, kind="ExternalInput")
        for t in range(NT)
    ]
    biasd = nc.dram_tensor("biasd", [1, NT * WTF], DT, kind="ExternalInput")
    out = nc.dram_tensor("out", [128, NGRP * 128], DT, kind="ExternalOutput")

    with tile.TileContext(nc) as tc:
        with (
            tc.tile_pool(name="xpool", bufs=1) as xpool,
            tc.tile_pool(name="wpool", bufs=1) as wpool,
            tc.tile_pool(name="opool", bufs=1) as opool,
            tc.tile_pool(name="pspool", bufs=4, space="PSUM") as pspool,
        ):
            stat_sb = xpool.tile([97, XF], DT, tag="stat")
            wt_all = wpool.tile([97, NT * WTF], DT, tag="wt")

            # x (2 chunks) on scalar queue, concurrent with the weight stream
            nc.scalar.dma_start(stat_sb[0:96, 0 : 3 * W * B], stat[:, 0 : 3 * W * B])
            nc.scalar.dma_start(stat_sb[0:96, 3 * W * B : XF], stat[:, 3 * W * B : XF])
            nc.gpsimd.memset(stat_sb[96:97, :], 1.0)

            # bias row first, then the 9 weight tiles back-to-back on sync
            nc.sync.dma_start(wt_all[96:97, :], biasd[:, :])
            for t in range(NT):
                nc.sync.dma_start(
                    wt_all[0:96, t * WTF : (t + 1) * WTF], wds[t][:, :]
                )

            out_sb = opool.tile([128, NGRP * 128], DT)

            for t in range(NT):
                ps = pspool.tile([128, 1024], F32)
                for gl in range(2):
                    gi = 2 * t + gl
                    rl, qg = divmod(gi, 3)
                    for m in range(4):
                        for kh in range(3):
                            for j in range(4):
                                q = qg * 16 + m * 4 + j
                                ll = gl * 16 + m * 4 + j
                                off = ((rl + kh) * W + q) * B
                                wo = t * WTF + (kh * TL + ll) * 32
                                nc.tensor.matmul(
                                    ps[
                                        32 * j : 32 * j + B,
                                        gl * 512 + m * 32 : gl * 512 + (m + 1) * 32,
                                    ],
                                    stat_sb[0:97, off : off + B],
                                    wt_all[0:97, wo : wo + 32],
                                    start=(kh == 0),
                                    stop=(kh == 2),
                                    skip_group_check=True,
                                    tile_position=(0, 32 * j),
                                )
                # move both groups' psum to SBUF, all on vector: scalar
                # copies chain behind vector's ($S>=2 cross-engine waits)
                # and PE tile t+4 gates on them via PSUM-bank reuse
                for j in range(4):
                    src = ps[32 * j : 32 * j + 32, 0:1024].rearrange(
                        "p (g f) -> p g f", g=2
                    )[:, :, 0:128]
                    dst = out_sb[
                        32 * j : 32 * j + 32, 256 * t : 256 * t + 256
                    ].rearrange("p (g f) -> p g f", g=2)
                    nc.vector.tensor_copy(dst, src)
                # stores: chunks of 3, 3, 2 tiles, then tile 8 alone so the
                # tail only waits on one 48KB store
                if t in (2, 5, 7, 8):
                    s0 = {2: 0, 5: 768, 7: 1536, 8: 2048}[t]
                    nc.gpsimd.dma_start(
                        out[:, s0 : 256 * (t + 1)],
                        out_sb[:, s0 : 256 * (t + 1)],
                    )
    nc.compile()
    return nc


def _shard(inputs):
    x = np.asarray(inputs["x"], np.float32)
    weight = np.asarray(inputs["weight"], np.float32)[0]  # (288, L, 32)
    bias = np.asarray(inputs["bias"], np.float32)[0]  # (32, 48, 48)
    xp = np.pad(x, ((0, 0), (0, 0), (1, 1), (1, 1)))  # (b, c, 50, 50)
    bias_t = bias.reshape(C_OUT, H * W).T  # (L, C_OUT)
    wflat = weight.reshape(C_IN, 3, 3, H * W, C_OUT)  # (c, kh, kw, l, o)

    in_maps = []
    for k in range(N_CORES):
        r0 = RP * k
        l0 = LP * k

        x3h = np.empty((3, C_IN, RP + 2, W, B), np.float32)
        for kw in range(3):
            x3h[kw] = xp[:, :, r0 : r0 + RP + 2, kw : kw + W].transpose(1, 2, 3, 0)
        stat = x3h.reshape(96, XF).astype(NPDT)

        # W tiles: [(kw c), (kh, lg, o)]; bias row separately, kh=0 block only
        wk = wflat[:, :, :, l0 : l0 + LP, :]  # (c, kh, kw, LP, o)
        wall = wk.transpose(2, 0, 1, 3, 4).reshape(96, 3, LP, C_OUT)
        m = {"stat": stat}
        for t in range(NT):
            m[f"w{t}"] = np.ascontiguousarray(
                wall[:, :, TL * t : TL * (t + 1), :].reshape(96, WTF)
            ).astype(NPDT)
        brow = np.zeros((NT, 3, TL, C_OUT), NPDT)
        brow[:, 0] = bias_t[l0 : l0 + LP, :].reshape(NT, TL, C_OUT).astype(NPDT)
        m["biasd"] = brow.reshape(1, NT * WTF)
        in_maps.append(m)
    return in_maps


def _get_nc():
    global _nc
    if _nc is None:
        _nc = _build()
    return _nc


def _gather(results):
    # out row 32j+b (b<8) holds y[b, o, r, q] at col gi*128 + m*32 + o,
    # with r = gi//3, q = (gi%3)*16 + m*4 + j
    y = np.empty((B, C_OUT, H, W), np.float32)
    for k in range(N_CORES):
        arr = results[k]["out"].astype(np.float32)
        arr = arr.reshape(4, 32, NGRP, 4, C_OUT)[:, 0:B]  # (j, b, gi, m, o)
        arr = arr.transpose(1, 4, 2, 3, 0)  # (b, o, gi, m, j)
        arr = arr.reshape(B, C_OUT, RP, 3, 4, 4)  # (b, o, r, qg, m, j)
        y[:, :, RP * k : RP * (k + 1), :] = arr.reshape(B, C_OUT, RP, W)
    return y


def kernel(**inputs):
    nc = _get_nc()
    res = run_bass_kernel_spmd(nc, _shard(inputs), list(range(N_CORES)))
    return _gather(res.results)
